# revision 23
# baseline (speedup 1.0000x reference)
"""DMP network kernel for Trainium2 (8 NeuronCores, pure data parallel).

Math: the reference is a 54->54 linear layer followed by a 301-step Euler
integration of a DMP (dynamic movement primitive). The phase variable xp and
hence the RBF activations psi are batch-independent, and the (y, z) scan is a
linear time-invariant recurrence driven by fx = (g - y0) * (w @ P_t). The
whole scan collapses to the closed form

    Y[b, d, t] = a_t * y0 + beta_t * g + (g - y0) * (w @ Q)[b, d, t]

with a, beta [T] and Q [N, T] computed on the host from c / sigma2 in float64.

Scaling a batch row of x by a per-row scalar commutes through any matmul, so
(g - y0) * (w @ Q) = (x_ext * dcol) @ (W2w.T @ Q) with x_ext = [x, 1] and
dcol = g - y0. The device pipeline per 128-row batch tile (x arrives
host-transposed as xT [55, batch], duplicated on partitions 0..54 / 64..118,
with ones planted at partitions 55,56 and 119,120):

  1. HBC matmul (per 4 tiles): hb [128, 512] = ch.T @ xT, where ch's columns
     replicate the dcol coefficient across partitions 0..54 (and 64..118 for
     DOF 1) and put the y0/g coefficients at partitions 55,56 / 119,120.
     So hb = [dcol0 x55 rows; y0_0; g_0; ...; dcol1 x55; y0_1; g_1] per batch.
  2. One VectorE multiply: mt [121, 128] = xin * hb  -> rows 0..54 carry
     x*dcol0, row 55,56 carry y0_0, g_0 (ones * hb), rows 64.. the DOF-1 copy.
  3. One matmul per DOF: Y_d [128, 302] = mt[d].T @ [A_d; a; beta]  -- the
     complete output tile in PSUM (A_d = W2w_d.T @ Q).
  4. Plain PSUM->SBUF copies (ScalarE for d0, VectorE for d1) + batched DMA.
"""

import os
import numpy as np

# -- problem constants (fixed by the reference) -------------------------------
N = 25
DOF = 2
TAU = 3.0
DT = 0.01
A_X = 2.0
A_Z = 48.0
B_Z = A_Z / 4.0
T = 301
D_IN = 54           # DOF * (N + 2)
B = 65536
N_CORES = 8
B_CORE = B // N_CORES          # 8192
P = 128                        # batch rows per tile
N_TILES = B_CORE // P          # 64
X_CHUNK = 4                    # tiles per input DMA
HB_CHUNK = 4                   # tiles per head-broadcast matmul
Y_CHUNK = 2                    # tiles per output DMA
D_PAD = 55                     # 54 features + ones row
T_PAD = 302                    # fp32r matmul needs an even moving-dim count
W_HI = 64                      # partition offset of the DOF-1 block
MT_H = 121                     # mt rows: 0..56 d0 block, 64..120 d1 block


# -- host-side closed-form constants ------------------------------------------
def _closed_form_consts(c, sigma2):
    """a [T], beta [T], Q [N, T] in float64."""
    c = np.asarray(c, np.float64)
    sigma2 = np.asarray(sigma2, np.float64)
    alpha = DT / TAU

    xp = np.empty(T)
    xp[0] = 1.0
    for t in range(T - 1):
        xp[t + 1] = xp[t] - (A_X * xp[t] / TAU) * DT
    psi = np.exp(-0.5 * (xp[:, None] - c[None, :]) ** 2 / sigma2[None, :])  # [T, N]
    S = psi.sum(1)
    Pmat = (psi * (xp / S)[:, None]).T                                      # [N, T]

    A = np.array([[1.0, alpha], [-alpha * A_Z * B_Z, 1.0 - alpha * A_Z]])
    a = np.empty(T)
    bvec = np.empty(T)
    M = np.eye(2)
    for t in range(T):
        a[t] = M[0, 0]
        bvec[t] = M[0, 1]
        M = A @ M
    beta = A_Z * B_Z * alpha * np.concatenate([[0.0], np.cumsum(bvec)[:-1]])

    H = np.zeros((T, T))
    for t in range(1, T):
        H[:t, t] = alpha * bvec[t - 1::-1]
    Q = Pmat @ H                                                            # [N, T]
    return a, beta, Q


def _host_inputs(x, W, b, c, sigma2, scale):
    """Build per-core input maps (numpy float32)."""
    a, beta, Q = _closed_form_consts(c, sigma2)

    W2 = np.asarray(W, np.float64) * np.asarray(scale, np.float64)[:, None]
    b2 = np.asarray(b, np.float64) * np.asarray(scale, np.float64)

    # w2e[:, j] = 55-vector [W2[j, :], b2[j]] -- the ones row carries the bias
    w2e = np.concatenate([W2.T, b2[None, :]], axis=0)       # [55, 54]

    # head-broadcast coefficients ch [55, 128]
    ch = np.zeros((D_PAD, P), np.float64)
    for d, lo in ((0, 0), (1, W_HI)):
        base = d * (N + 2)
        dc = w2e[:, base + 1] - w2e[:, base]
        ch[:, lo:lo + D_PAD] = dc[:, None]
        ch[:, lo + D_PAD] = w2e[:, base]          # y0_d coeff
        ch[:, lo + D_PAD + 1] = w2e[:, base + 1]  # g_d coeff
    ch = np.ascontiguousarray(ch.astype(np.float32))

    # Y-matmul coefficients cy [128, 604]: rows 0..56 d0, rows 64..120 d1
    cy = np.zeros((P, DOF * T_PAD), np.float64)
    for d, lo in ((0, 0), (1, W_HI)):
        base = d * (N + 2)
        cy[lo:lo + D_PAD, d * T_PAD:d * T_PAD + T] = w2e[:, base + 2:base + 2 + N] @ Q
        cy[lo + D_PAD, d * T_PAD:d * T_PAD + T] = a
        cy[lo + D_PAD + 1, d * T_PAD:d * T_PAD + T] = beta
    cy = np.ascontiguousarray(cy.astype(np.float32))

    # host-transposed x image [64, B]: x on rows 0..53, bias-ones row 54,
    # head pass-through ones rows 55,56, zeros 57..63. The device duplicates
    # rows 0..63 onto partitions 64..127 (DOF-1 block) with a GpSimd copy.
    xT = np.zeros((W_HI, B), np.float32)
    xT[:D_IN] = np.asarray(x, np.float32).T
    xT[D_IN] = 1.0
    xT[D_PAD:D_PAD + 2] = 1.0

    in_maps = []
    for ci in range(N_CORES):
        in_maps.append({
            "x": np.ascontiguousarray(xT[:, ci * B_CORE:(ci + 1) * B_CORE]),
            "ch": ch,
            "cy": cy,
        })
    return in_maps


# -- bass program --------------------------------------------------------------
_NC_CACHE = None


def _build_program():
    global _NC_CACHE
    if _NC_CACHE is not None:
        return _NC_CACHE

    import concourse.bacc as bacc
    import concourse.tile as tile
    from concourse import mybir
    from contextlib import ExitStack

    f32 = mybir.dt.float32
    f32r = mybir.dt.float32r

    nc = bacc.Bacc(
        "TRN2",
        target_bir_lowering=False,
        debug=False,
        num_devices=N_CORES,
    )
    x_d = nc.declare_dram_parameter("x", [W_HI, B_CORE], f32r, isOutput=False)
    ch_d = nc.declare_dram_parameter("ch", [D_PAD, P], f32r, isOutput=False)
    cy_d = nc.declare_dram_parameter("cy", [P, DOF * T_PAD], f32r, isOutput=False)
    y_d = nc.declare_dram_parameter("y", [B_CORE, DOF * T], f32, isOutput=True)

    with tile.TileContext(nc) as tc, ExitStack() as ctx:
        consts = ctx.enter_context(tc.tile_pool(name="consts", bufs=1))
        xin_p = ctx.enter_context(tc.tile_pool(name="xin", bufs=4))
        mt_p = ctx.enter_context(tc.tile_pool(name="mt", bufs=8))
        yout_p = ctx.enter_context(tc.tile_pool(name="yout", bufs=8))
        hb_p = ctx.enter_context(tc.tile_pool(name="hb", bufs=2, space="PSUM"))
        ps_p = ctx.enter_context(tc.tile_pool(name="ps", bufs=6, space="PSUM"))

        ch_sb = consts.tile([D_PAD, P], f32r)
        nc.sync.dma_start(ch_sb[:], ch_d[:])
        cy_sb = consts.tile([P, DOF * T_PAD], f32r)
        nc.sync.dma_start(cy_sb[:], cy_d[:])

        y_view = y_d.rearrange("(nt p) f -> nt p f", p=P)      # [64, 128, 602]

        ysb = None
        for ci in range(N_TILES // X_CHUNK):
            CW = X_CHUNK * P
            xin = xin_p.tile([P, CW], f32r)
            src = x_d[:, ci * CW:(ci + 1) * CW]
            # ScalarE HWDGE queue: separate FIFO from the output DMAs
            nc.scalar.dma_start(xin[0:W_HI, :], src)
            # duplicate the block for the DOF-1 partitions on the idle GpSimd
            nc.gpsimd.tensor_copy(xin[W_HI:P, :], xin[0:W_HI, :])

            for j in range(X_CHUNK):
                i = ci * X_CHUNK + j
                jc = j * P

                if j % HB_CHUNK == 0:
                    HW_ = HB_CHUNK * P
                    hb = hb_p.tile([P, HW_], f32)
                    nc.tensor.matmul(hb[:], ch_sb[:], xin[0:D_PAD, jc:jc + HW_],
                                     start=True, stop=True)
                hcol = (j % HB_CHUNK) * P

                # mt rows: [x*dcol0 (55); y0_0; g_0; 0...; x*dcol1; y0_1; g_1]
                mt = mt_p.tile([MT_H, P], f32r, tag="mt")
                nc.vector.tensor_mul(mt[:], xin[0:MT_H, jc:jc + P],
                                     hb[0:MT_H, hcol:hcol + P])

                ps_y0 = ps_p.tile([P, T_PAD], f32, tag="ps")
                ps_y1 = ps_p.tile([P, T_PAD], f32, tag="ps")
                nc.tensor.matmul(ps_y0[:], mt[0:D_PAD + 2, :],
                                 cy_sb[0:D_PAD + 2, 0:T_PAD],
                                 start=True, stop=True)
                nc.tensor.matmul(ps_y1[:], mt[W_HI:MT_H, :],
                                 cy_sb[W_HI:MT_H, T_PAD:2 * T_PAD],
                                 start=True, stop=True)

                if j % Y_CHUNK == 0:
                    ysb = yout_p.tile([P, Y_CHUNK, DOF * T], f32)
                yrow = ysb[:, j % Y_CHUNK]
                # rotate copy assignment: DVE also carries the mt multiply, so
                # ScalarE takes both halves every third tile
                nc.scalar.copy(yrow[:, 0:T], ps_y0[:, 0:T])
                if i % 4 == 0:
                    nc.scalar.copy(yrow[:, T:2 * T], ps_y1[:, 0:T])
                else:
                    nc.vector.tensor_copy(yrow[:, T:2 * T], ps_y1[:, 0:T])

                if j % Y_CHUNK == Y_CHUNK - 1:
                    i0 = i - (Y_CHUNK - 1)
                    dst = y_view[i0:i0 + Y_CHUNK].rearrange("n p f -> p n f")
                    nc.sync.dma_start(dst, ysb[:])

    nc.compile()
    _NC_CACHE = nc
    return nc


_LAST_RESULTS = None


def kernel(x, W, b, c, sigma2, scale):
    global _LAST_RESULTS
    from concourse.bass_utils import run_bass_kernel_spmd

    assert x.shape == (B, D_IN), x.shape
    nc = _build_program()
    in_maps = _host_inputs(x, W, b, c, sigma2, scale)
    res = run_bass_kernel_spmd(nc, in_maps, list(range(N_CORES)))
    _LAST_RESULTS = res
    out = np.concatenate([res.results[ci]["y"] for ci in range(N_CORES)], axis=0)
    return out.astype(np.float32)


# revision 24
# speedup vs baseline: 1.0166x; 1.0166x over previous
"""DMP network kernel for Trainium2 (8 NeuronCores, pure data parallel).

Math: the reference is a 54->54 linear layer followed by a 301-step Euler
integration of a DMP (dynamic movement primitive). The phase variable xp and
hence the RBF activations psi are batch-independent, and the (y, z) scan is a
linear time-invariant recurrence driven by fx = (g - y0) * (w @ P_t). The
whole scan collapses to the closed form

    Y[b, d, t] = a_t * y0 + beta_t * g + (g - y0) * (w @ Q)[b, d, t]

with a, beta [T] and Q [N, T] computed on the host from c / sigma2 in float64.

Scaling a batch row of x by a per-row scalar commutes through any matmul, so
(g - y0) * (w @ Q) = (x_ext * dcol) @ (W2w.T @ Q) with x_ext = [x, 1] and
dcol = g - y0. The device pipeline per 128-row batch tile (x arrives
host-transposed as xT [55, batch], duplicated on partitions 0..54 / 64..118,
with ones planted at partitions 55,56 and 119,120):

  1. HBC matmul (per 4 tiles): hb [128, 512] = ch.T @ xT, where ch's columns
     replicate the dcol coefficient across partitions 0..54 (and 64..118 for
     DOF 1) and put the y0/g coefficients at partitions 55,56 / 119,120.
     So hb = [dcol0 x55 rows; y0_0; g_0; ...; dcol1 x55; y0_1; g_1] per batch.
  2. One VectorE multiply: mt [121, 128] = xin * hb  -> rows 0..54 carry
     x*dcol0, row 55,56 carry y0_0, g_0 (ones * hb), rows 64.. the DOF-1 copy.
  3. One matmul per DOF: Y_d [128, 302] = mt[d].T @ [A_d; a; beta]  -- the
     complete output tile in PSUM (A_d = W2w_d.T @ Q).
  4. Plain PSUM->SBUF copies (ScalarE for d0, VectorE for d1) + batched DMA.
"""

import os
import numpy as np

# -- problem constants (fixed by the reference) -------------------------------
N = 25
DOF = 2
TAU = 3.0
DT = 0.01
A_X = 2.0
A_Z = 48.0
B_Z = A_Z / 4.0
T = 301
D_IN = 54           # DOF * (N + 2)
B = 65536
N_CORES = 8
B_CORE = B // N_CORES          # 8192
P = 128                        # batch rows per tile
N_TILES = B_CORE // P          # 64
X_CHUNK = 8                    # tiles per input DMA
HB_CHUNK = 4                   # tiles per head-broadcast matmul
Y_CHUNK = 2                    # tiles per output DMA
D_PAD = 55                     # 54 features + ones row
T_PAD = 302                    # fp32r matmul needs an even moving-dim count
W_HI = 64                      # partition offset of the DOF-1 block
MT_H = 121                     # mt rows: 0..56 d0 block, 64..120 d1 block


# -- host-side closed-form constants ------------------------------------------
def _closed_form_consts(c, sigma2):
    """a [T], beta [T], Q [N, T] in float64."""
    c = np.asarray(c, np.float64)
    sigma2 = np.asarray(sigma2, np.float64)
    alpha = DT / TAU

    xp = np.empty(T)
    xp[0] = 1.0
    for t in range(T - 1):
        xp[t + 1] = xp[t] - (A_X * xp[t] / TAU) * DT
    psi = np.exp(-0.5 * (xp[:, None] - c[None, :]) ** 2 / sigma2[None, :])  # [T, N]
    S = psi.sum(1)
    Pmat = (psi * (xp / S)[:, None]).T                                      # [N, T]

    A = np.array([[1.0, alpha], [-alpha * A_Z * B_Z, 1.0 - alpha * A_Z]])
    a = np.empty(T)
    bvec = np.empty(T)
    M = np.eye(2)
    for t in range(T):
        a[t] = M[0, 0]
        bvec[t] = M[0, 1]
        M = A @ M
    beta = A_Z * B_Z * alpha * np.concatenate([[0.0], np.cumsum(bvec)[:-1]])

    H = np.zeros((T, T))
    for t in range(1, T):
        H[:t, t] = alpha * bvec[t - 1::-1]
    Q = Pmat @ H                                                            # [N, T]
    return a, beta, Q


def _host_inputs(x, W, b, c, sigma2, scale):
    """Build per-core input maps (numpy float32)."""
    a, beta, Q = _closed_form_consts(c, sigma2)

    W2 = np.asarray(W, np.float64) * np.asarray(scale, np.float64)[:, None]
    b2 = np.asarray(b, np.float64) * np.asarray(scale, np.float64)

    # w2e[:, j] = 55-vector [W2[j, :], b2[j]] -- the ones row carries the bias
    w2e = np.concatenate([W2.T, b2[None, :]], axis=0)       # [55, 54]

    # head-broadcast coefficients ch [55, 128]
    ch = np.zeros((D_PAD, P), np.float64)
    for d, lo in ((0, 0), (1, W_HI)):
        base = d * (N + 2)
        dc = w2e[:, base + 1] - w2e[:, base]
        ch[:, lo:lo + D_PAD] = dc[:, None]
        ch[:, lo + D_PAD] = w2e[:, base]          # y0_d coeff
        ch[:, lo + D_PAD + 1] = w2e[:, base + 1]  # g_d coeff
    ch = np.ascontiguousarray(ch.astype(np.float32))

    # Y-matmul coefficients cy [128, 604]: rows 0..56 d0, rows 64..120 d1
    cy = np.zeros((P, DOF * T_PAD), np.float64)
    for d, lo in ((0, 0), (1, W_HI)):
        base = d * (N + 2)
        cy[lo:lo + D_PAD, d * T_PAD:d * T_PAD + T] = w2e[:, base + 2:base + 2 + N] @ Q
        cy[lo + D_PAD, d * T_PAD:d * T_PAD + T] = a
        cy[lo + D_PAD + 1, d * T_PAD:d * T_PAD + T] = beta
    cy = np.ascontiguousarray(cy.astype(np.float32))

    # host-transposed x image [64, B]: x on rows 0..53, bias-ones row 54,
    # head pass-through ones rows 55,56, zeros 57..63. The device duplicates
    # rows 0..63 onto partitions 64..127 (DOF-1 block) with a GpSimd copy.
    xT = np.zeros((W_HI, B), np.float32)
    xT[:D_IN] = np.asarray(x, np.float32).T
    xT[D_IN] = 1.0
    xT[D_PAD:D_PAD + 2] = 1.0

    in_maps = []
    for ci in range(N_CORES):
        in_maps.append({
            "x": np.ascontiguousarray(xT[:, ci * B_CORE:(ci + 1) * B_CORE]),
            "ch": ch,
            "cy": cy,
        })
    return in_maps


# -- bass program --------------------------------------------------------------
_NC_CACHE = None


def _build_program():
    global _NC_CACHE
    if _NC_CACHE is not None:
        return _NC_CACHE

    import concourse.bacc as bacc
    import concourse.tile as tile
    from concourse import mybir
    from contextlib import ExitStack

    f32 = mybir.dt.float32
    f32r = mybir.dt.float32r

    nc = bacc.Bacc(
        "TRN2",
        target_bir_lowering=False,
        debug=False,
        num_devices=N_CORES,
    )
    x_d = nc.declare_dram_parameter("x", [W_HI, B_CORE], f32r, isOutput=False)
    ch_d = nc.declare_dram_parameter("ch", [D_PAD, P], f32r, isOutput=False)
    cy_d = nc.declare_dram_parameter("cy", [P, DOF * T_PAD], f32r, isOutput=False)
    y_d = nc.declare_dram_parameter("y", [B_CORE, DOF * T], f32, isOutput=True)

    with tile.TileContext(nc) as tc, ExitStack() as ctx:
        consts = ctx.enter_context(tc.tile_pool(name="consts", bufs=1))
        xin_p = ctx.enter_context(tc.tile_pool(name="xin", bufs=4))
        mt_p = ctx.enter_context(tc.tile_pool(name="mt", bufs=8))
        yout_p = ctx.enter_context(tc.tile_pool(name="yout", bufs=8))
        hb_p = ctx.enter_context(tc.tile_pool(name="hb", bufs=2, space="PSUM"))
        ps_p = ctx.enter_context(tc.tile_pool(name="ps", bufs=6, space="PSUM"))

        ch_sb = consts.tile([D_PAD, P], f32r)
        nc.sync.dma_start(ch_sb[:], ch_d[:])
        cy_sb = consts.tile([P, DOF * T_PAD], f32r)
        nc.sync.dma_start(cy_sb[:], cy_d[:])

        y_view = y_d.rearrange("(nt p) f -> nt p f", p=P)      # [64, 128, 602]

        ysb = None
        for ci in range(N_TILES // X_CHUNK):
            CW = X_CHUNK * P
            xin = xin_p.tile([P, CW], f32r)
            src = x_d[:, ci * CW:(ci + 1) * CW]
            # ScalarE HWDGE queue: separate FIFO from the output DMAs.
            # First chunk lands in halves so tile 0 starts sooner; the DOF-1
            # partition block is duplicated on the idle GpSimd per half.
            H = CW // 2 if ci == 0 else CW
            for c0 in range(0, CW, H):
                nc.scalar.dma_start(xin[0:W_HI, c0:c0 + H], src[:, c0:c0 + H])
                nc.gpsimd.tensor_copy(xin[W_HI:P, c0:c0 + H],
                                      xin[0:W_HI, c0:c0 + H])

            for j in range(X_CHUNK):
                i = ci * X_CHUNK + j
                jc = j * P

                if j % HB_CHUNK == 0:
                    HW_ = HB_CHUNK * P
                    hb = hb_p.tile([P, HW_], f32)
                    nc.tensor.matmul(hb[:], ch_sb[:], xin[0:D_PAD, jc:jc + HW_],
                                     start=True, stop=True)
                hcol = (j % HB_CHUNK) * P

                # mt rows: [x*dcol0 (55); y0_0; g_0; 0...; x*dcol1; y0_1; g_1]
                mt = mt_p.tile([MT_H, P], f32r, tag="mt")
                nc.vector.tensor_mul(mt[:], xin[0:MT_H, jc:jc + P],
                                     hb[0:MT_H, hcol:hcol + P])

                ps_y0 = ps_p.tile([P, T_PAD], f32, tag="ps")
                ps_y1 = ps_p.tile([P, T_PAD], f32, tag="ps")
                nc.tensor.matmul(ps_y0[:], mt[0:D_PAD + 2, :],
                                 cy_sb[0:D_PAD + 2, 0:T_PAD],
                                 start=True, stop=True)
                nc.tensor.matmul(ps_y1[:], mt[W_HI:MT_H, :],
                                 cy_sb[W_HI:MT_H, T_PAD:2 * T_PAD],
                                 start=True, stop=True)

                if j % Y_CHUNK == 0:
                    ysb = yout_p.tile([P, Y_CHUNK, DOF * T], f32)
                yrow = ysb[:, j % Y_CHUNK]
                # rotate copy assignment: DVE also carries the mt multiply, so
                # ScalarE takes both halves every third tile
                nc.scalar.copy(yrow[:, 0:T], ps_y0[:, 0:T])
                if i % 4 == 0:
                    nc.scalar.copy(yrow[:, T:2 * T], ps_y1[:, 0:T])
                else:
                    nc.vector.tensor_copy(yrow[:, T:2 * T], ps_y1[:, 0:T])

                if j % Y_CHUNK == Y_CHUNK - 1:
                    i0 = i - (Y_CHUNK - 1)
                    dst = y_view[i0:i0 + Y_CHUNK].rearrange("n p f -> p n f")
                    nc.sync.dma_start(dst, ysb[:])

    nc.compile()
    _NC_CACHE = nc
    return nc


_LAST_RESULTS = None


def kernel(x, W, b, c, sigma2, scale):
    global _LAST_RESULTS
    from concourse.bass_utils import run_bass_kernel_spmd

    assert x.shape == (B, D_IN), x.shape
    nc = _build_program()
    in_maps = _host_inputs(x, W, b, c, sigma2, scale)
    res = run_bass_kernel_spmd(nc, in_maps, list(range(N_CORES)))
    _LAST_RESULTS = res
    out = np.concatenate([res.results[ci]["y"] for ci in range(N_CORES)], axis=0)
    return out.astype(np.float32)
